# revision 19
# baseline (speedup 1.0000x reference)
"""Trainium2 Bass kernel for nn_BackProjectionLoss.

Computes mean(|bicubic_downsample(output, 512->128) - input|) where the
downsample is the MATLAB-style antialiased cubic (Keys a=-0.5) separable
filter from the reference, with symmetric padding.

Strategy (pure data parallel, 8 cores, 12 images of 512x512 each):
  The separable downsample is Y2 = D @ X @ D^T with a dense 128x512
  matrix D (symmetric padding folded in).  All matmuls run in fp16
  (inputs in [0,1); fp16 keeps ~1e-7 relative error on the final scalar)
  with fp32 PSUM accumulation, arranged so NO transposes are needed:

    pass 1: Y1T[c,i] = sum_k X[k,c] * D[i,k]
            lhsT = fp16 image chunk  [k=128, c_block=128]  (PE weights)
            rhs  = D^T chunk         [k=128, i=128]        (static)
    pass 2: Y2[i,j] = sum_c Y1T[c,i] * D[j,c]
            lhsT = Y1T chunk [c=128, i=128], rhs = same D^T chunks
    compare: Y2 lands natural-layout; |Y2 - In| via DVE sub + abs-reduce.

  Per-core output: per-partition partial sums [128, n_groups]; the host
  sums in float64 and divides by the element count.
"""

import numpy as np
from math import ceil

import concourse.mybir as mybir
import concourse.tile as tile
from concourse import bacc
from concourse.bass_utils import run_bass_kernel_spmd

F32 = mybir.dt.float32
F16 = mybir.dt.float16

N_CORES = 8
B, C, H, W = 32, 3, 512, 512
h, w = 128, 128
N_IMG = B * C                     # 96 images
IMG_PER_CORE = N_IMG // N_CORES   # 12
GROUP = 4                         # images per compare batch
N_GROUPS = IMG_PER_CORE // GROUP  # 3


def _downsample_matrix():
    """128x512 antialiased-cubic downsample matrix, symmetric pad folded in.

    Mirrors reference._make_kernel(512, 128) + jnp.pad(mode="symmetric") +
    strided valid conv.
    """
    in_len, out_len = H, h
    scale = out_len / in_len
    klen = 4.0 / scale
    x = np.array([1, out_len], dtype=np.float64)
    u = x / scale + 0.5 * (1.0 - 1.0 / scale)
    left = np.floor(u - klen / 2.0)
    p = int(ceil(klen)) + 2
    ind = left[:, None] + np.arange(p) - 1
    indices = ind.astype(np.int64)
    xx = (u[:, None] - indices - 1) * scale
    ax = np.abs(xx)
    ax2 = ax * ax
    ax3 = ax2 * ax
    wgt = (1.5 * ax3 - 2.5 * ax2 + 1.0) * (ax <= 1) + \
          (-0.5 * ax3 + 2.5 * ax2 - 4.0 * ax + 2.0) * ((ax > 1) & (ax <= 2))
    wgt = wgt * scale
    wgt = wgt / wgt.sum(axis=1, keepdims=True)
    keep = np.nonzero(np.any(wgt, axis=0))[0]
    wgt = wgt[:, keep]
    indices = indices[:, keep]
    assert np.all(wgt[0] == wgt[-1])
    pad_l = int(np.where(indices[0] == 0)[0][0])
    taps = wgt[0].astype(np.float32)
    L = taps.shape[0]
    stride = in_len // out_len

    D = np.zeros((out_len, in_len), dtype=np.float64)
    for i in range(out_len):
        for t in range(L):
            s = stride * i + t - pad_l
            if s < 0:
                s = -s - 1
            if s >= in_len:
                s = 2 * in_len - 1 - s
            D[i, s] += float(taps[t])
    return D.astype(np.float32)


def _build_program():
    Dp = _downsample_matrix()                      # [128, 512]
    # dmat[kc, k, i] = D[i, kc*128 + k]  (D^T in 4 chunks of 128 rows)
    dmat = np.ascontiguousarray(
        Dp.T.reshape(4, 128, 128)).astype(np.float16)

    nc = bacc.Bacc("TRN2", target_bir_lowering=False, debug=False,
                   num_devices=N_CORES)
    # x is host-prepared fp16, swizzled as [img, k_local, kc, c] so that
    # x[im, p] is 4 KiB contiguous per partition and x_sb[p, kc, c] =
    # X[kc*128 + p, c] (pass-1 weight-chunk layout).
    x_d = nc.dram_tensor("x", [IMG_PER_CORE, 128, 4, W], F16,
                         kind="ExternalInput")
    # "inp" is host-pre-transposed: inp[g, j, i] = input_image_g[i, j]
    in_d = nc.dram_tensor("inp", [IMG_PER_CORE, h, w], F32,
                          kind="ExternalInput")
    out_d = nc.dram_tensor("out", [128, N_GROUPS], F32, kind="ExternalOutput")
    dmat_d = nc.inline_tensor(dmat, "dmat")        # [4, 128, 128] fp16

    # nonzero i-band of D^T chunk kc: D[i, s] != 0 only for 4i-6 <= s <= 4i+9.
    # kc==0 stays dense (it initializes the whole 128-col block, writing the
    # zero columns too, so the banded kc>=1 accumulates land on uniformly
    # initialized PSUM bytes).
    bands = [(0, 128)] * 4

    with tile.TileContext(nc) as tc:
        with (
            tc.tile_pool(name="const", bufs=1) as const_pool,
            tc.tile_pool(name="x16p", bufs=2) as x16_pool,
            tc.tile_pool(name="y1tbp", bufs=2) as y1tb_pool,
            tc.tile_pool(name="inb", bufs=2) as in_pool,
            tc.tile_pool(name="diff", bufs=2) as diff_pool,
            tc.tile_pool(name="y1tp", bufs=3, space="PSUM") as y1t_psum,
            tc.tile_pool(name="y2p", bufs=2, space="PSUM") as y2_psum,
        ):
            dt16 = const_pool.tile([128, 4, 128], F16)
            nc.sync.dma_start(out=dt16,
                                in_=dmat_d.ap().rearrange("c k i -> k c i"))
            acc = const_pool.tile([128, N_GROUPS], F32)

            for grp in range(N_GROUPS):
                # Y1T batch: [c_local, cc, g, i] fp16 (pass-2 moving operand)
                y1tb = y1tb_pool.tile([128, 4, GROUP, 128], F16)
                in_sb = None

                for ig in range(GROUP):
                    im = grp * GROUP + ig
                    x16 = x16_pool.tile([128, 4, 512], F16)
                    nc.sync.dma_start(out=x16, in_=x_d[im])

                    # pass 1: Y1T[c, i], image chunks as PE weights.
                    # rhs restricted to the nonzero i-band of each D^T chunk;
                    # bands of consecutive kc overlap, which accumulates
                    # correctly via PSUM has_written semantics (kc==0 pends
                    # the whole bank; fresh bytes overwrite, seen accumulate).
                    y1t_ps = y1t_psum.tile([128, 512], F32)
                    for cc in range(4):
                        base = cc * 128
                        for kc in range(4):
                            lo, hi = bands[kc]
                            nc.tensor.matmul(
                                y1t_ps[:, base + lo:base + hi],
                                x16[:, kc, base:base + 128],
                                dt16[:, kc, lo:hi],
                                start=(kc == 0), stop=(kc == 3))
                    # evict to the group batch (ACT engine; casts to fp16)
                    nc.scalar.copy(
                        out=y1tb[:, :, ig, :],
                        in_=y1t_ps.rearrange("p (c i) -> p c i", c=4))
                    if ig == 0:
                        # In^T batch for the group, issued after the group's
                        # first X load so it doesn't delay the weight stream
                        in_sb = in_pool.tile([128, GROUP, 128], F32)
                        nc.sync.dma_start(
                            out=in_sb,
                            in_=in_d[grp * GROUP:(grp + 1) * GROUP].rearrange(
                                "g j i -> j g i"))

                # pass 2 batched over the group: Y2T[j, (g i)]
                y2t_ps = y2_psum.tile([128, GROUP * 128], F32)
                for cc in range(4):
                    nc.tensor.matmul(
                        y2t_ps,
                        dt16[:, cc, :],
                        y1tb[:, cc, :, :].rearrange("p g i -> p (g i)"),
                        start=(cc == 0), stop=(cc == 3))

                diff_sb = diff_pool.tile([128, GROUP * 128], F32)
                nc.vector.tensor_sub(
                    diff_sb, y2t_ps,
                    in_sb.rearrange("p g i -> p (g i)"))
                nc.vector.tensor_reduce(
                    out=acc[:, grp:grp + 1], in_=diff_sb,
                    axis=mybir.AxisListType.X, op=mybir.AluOpType.add,
                    apply_absolute_value=True)

            nc.sync.dma_start(out=out_d.ap(), in_=acc)

    nc.compile()
    return nc


_PROGRAM = None


def _get_program():
    global _PROGRAM
    if _PROGRAM is None:
        _PROGRAM = _build_program()
    return _PROGRAM


def _shard_inputs(input, output):
    # fp16 quantization of X (identical numerics to an on-device cast)
    # + swizzle to [img, k_local, kc, c] for contiguous per-partition DMA.
    xs = np.asarray(output, dtype=np.float32).reshape(N_IMG, 4, 128, W)
    xs = np.ascontiguousarray(
        xs.transpose(0, 2, 1, 3)).astype(np.float16)
    # pre-transpose the small target images so the kernel compares Y2^T
    ins = np.ascontiguousarray(np.swapaxes(
        np.asarray(input, dtype=np.float32).reshape(N_IMG, h, w), 1, 2))
    in_maps = []
    for c in range(N_CORES):
        sl = slice(c * IMG_PER_CORE, (c + 1) * IMG_PER_CORE)
        in_maps.append({
            "x": np.ascontiguousarray(xs[sl]),
            "inp": np.ascontiguousarray(ins[sl]),
        })
    return in_maps


def _run(input, output, **kwargs):
    nc = _get_program()
    in_maps = _shard_inputs(input, output)
    res = run_bass_kernel_spmd(nc, in_maps, core_ids=list(range(N_CORES)),
                               **kwargs)
    total = 0.0
    for r in res.results:
        total += r["out"].astype(np.float64).sum()
    mean = total / float(N_IMG * h * w)
    return np.float32(mean), res


def kernel(input, output):
    val, _ = _run(input, output)
    return np.asarray(val, dtype=np.float32)


# revision 20
# speedup vs baseline: 1.3392x; 1.3392x over previous
"""Trainium2 Bass kernel for nn_BackProjectionLoss.

Computes mean(|bicubic_downsample(output, 512->128) - input|) where the
downsample is the MATLAB-style antialiased cubic (Keys a=-0.5) separable
filter from the reference, with symmetric padding.

Strategy (pure data parallel, 8 cores, 12 images of 512x512 each):
  The separable downsample is Y2 = D @ X @ D^T with a dense 128x512
  matrix D (symmetric padding folded in).  All matmuls run in fp16
  (inputs in [0,1); fp16 keeps ~1e-7 relative error on the final scalar)
  with fp32 PSUM accumulation, arranged so NO transposes are needed:

    pass 1: Y1T[c,i] = sum_k X[k,c] * D[i,k]
            lhsT = fp16 image chunk  [k=128, c_block=128]  (PE weights)
            rhs  = D^T chunk         [k=128, i=128]        (static)
    pass 2: Y2[i,j] = sum_c Y1T[c,i] * D[j,c]
            lhsT = Y1T chunk [c=128, i=128], rhs = same D^T chunks
    compare: Y2 lands natural-layout; |Y2 - In| via DVE sub + abs-reduce.

  Per-core output: per-partition partial sums [128, n_groups]; the host
  sums in float64 and divides by the element count.
"""

import numpy as np
from math import ceil

import concourse.mybir as mybir
import concourse.tile as tile
from concourse import bacc
from concourse.bass_utils import run_bass_kernel_spmd

F32 = mybir.dt.float32
F16 = mybir.dt.float16

N_CORES = 8
B, C, H, W = 32, 3, 512, 512
h, w = 128, 128
N_IMG = B * C                     # 96 images
IMG_PER_CORE = N_IMG // N_CORES   # 12
GROUP = 4                         # images per compare batch
N_GROUPS = IMG_PER_CORE // GROUP  # 3


def _downsample_matrix():
    """128x512 antialiased-cubic downsample matrix, symmetric pad folded in.

    Mirrors reference._make_kernel(512, 128) + jnp.pad(mode="symmetric") +
    strided valid conv.
    """
    in_len, out_len = H, h
    scale = out_len / in_len
    klen = 4.0 / scale
    x = np.array([1, out_len], dtype=np.float64)
    u = x / scale + 0.5 * (1.0 - 1.0 / scale)
    left = np.floor(u - klen / 2.0)
    p = int(ceil(klen)) + 2
    ind = left[:, None] + np.arange(p) - 1
    indices = ind.astype(np.int64)
    xx = (u[:, None] - indices - 1) * scale
    ax = np.abs(xx)
    ax2 = ax * ax
    ax3 = ax2 * ax
    wgt = (1.5 * ax3 - 2.5 * ax2 + 1.0) * (ax <= 1) + \
          (-0.5 * ax3 + 2.5 * ax2 - 4.0 * ax + 2.0) * ((ax > 1) & (ax <= 2))
    wgt = wgt * scale
    wgt = wgt / wgt.sum(axis=1, keepdims=True)
    keep = np.nonzero(np.any(wgt, axis=0))[0]
    wgt = wgt[:, keep]
    indices = indices[:, keep]
    assert np.all(wgt[0] == wgt[-1])
    pad_l = int(np.where(indices[0] == 0)[0][0])
    taps = wgt[0].astype(np.float32)
    L = taps.shape[0]
    stride = in_len // out_len

    D = np.zeros((out_len, in_len), dtype=np.float64)
    for i in range(out_len):
        for t in range(L):
            s = stride * i + t - pad_l
            if s < 0:
                s = -s - 1
            if s >= in_len:
                s = 2 * in_len - 1 - s
            D[i, s] += float(taps[t])
    return D.astype(np.float32)


def _build_program():
    Dp = _downsample_matrix()                      # [128, 512]
    # dmat[kc, k, i] = D[i, kc*128 + k]  (D^T in 4 chunks of 128 rows)
    dmat = np.ascontiguousarray(
        Dp.T.reshape(4, 128, 128)).astype(np.float16)

    nc = bacc.Bacc("TRN2", target_bir_lowering=False, debug=False,
                   num_devices=N_CORES)
    # x is host-prepared fp16, swizzled as [img, k_local, kc, c] so that
    # x[im, p] is 4 KiB contiguous per partition and x_sb[p, kc, c] =
    # X[kc*128 + p, c] (pass-1 weight-chunk layout).
    x_d = nc.dram_tensor("x", [IMG_PER_CORE, 128, 4, W], F16,
                         kind="ExternalInput")
    # "inp" is host-pre-transposed: inp[g, j, i] = input_image_g[i, j]
    in_d = nc.dram_tensor("inp", [IMG_PER_CORE, h, w], F32,
                          kind="ExternalInput")
    out_d = nc.dram_tensor("out", [128, N_GROUPS], F32, kind="ExternalOutput")
    dmat_d = nc.inline_tensor(dmat, "dmat")        # [4, 128, 128] fp16

    # nonzero i-band of D^T chunk kc: D[i, s] != 0 only for 4i-6 <= s <= 4i+9.
    # kc==0 stays dense (it initializes the whole 128-col block, writing the
    # zero columns too, so the banded kc>=1 accumulates land on uniformly
    # initialized PSUM bytes).
    bands = [(0, 128)] * 4

    with tile.TileContext(nc) as tc:
        with (
            tc.tile_pool(name="const", bufs=1) as const_pool,
            tc.tile_pool(name="x16p", bufs=4) as x16_pool,
            tc.tile_pool(name="y1tbp", bufs=3) as y1tb_pool,
            tc.tile_pool(name="inb", bufs=2) as in_pool,
            tc.tile_pool(name="diff", bufs=2) as diff_pool,
            tc.tile_pool(name="y1tp", bufs=4, space="PSUM") as y1t_psum,
            tc.tile_pool(name="y2p", bufs=2, space="PSUM") as y2_psum,
        ):
            dt16 = const_pool.tile([128, 4, 128], F16)
            nc.sync.dma_start(out=dt16,
                                in_=dmat_d.ap().rearrange("c k i -> k c i"))
            acc = const_pool.tile([128, N_GROUPS], F32)

            for grp in range(N_GROUPS):
                # Y1T batch: [c_local, cc, g, i] fp16 (pass-2 moving operand)
                y1tb = y1tb_pool.tile([128, 4, GROUP, 128], F16)
                in_sb = None

                for ig in range(GROUP):
                    im = grp * GROUP + ig
                    x16 = x16_pool.tile([128, 4, 512], F16)
                    nc.sync.dma_start(out=x16, in_=x_d[im])

                    # pass 1: Y1T[c, i], image chunks as PE weights.
                    # rhs restricted to the nonzero i-band of each D^T chunk;
                    # bands of consecutive kc overlap, which accumulates
                    # correctly via PSUM has_written semantics (kc==0 pends
                    # the whole bank; fresh bytes overwrite, seen accumulate).
                    y1t_ps = y1t_psum.tile([128, 512], F32)
                    for cc in range(4):
                        base = cc * 128
                        for kc in range(4):
                            lo, hi = bands[kc]
                            nc.tensor.matmul(
                                y1t_ps[:, base + lo:base + hi],
                                x16[:, kc, base:base + 128],
                                dt16[:, kc, lo:hi],
                                start=(kc == 0), stop=(kc == 3))
                    # evict to the group batch (ACT engine; casts to fp16)
                    nc.scalar.copy(
                        out=y1tb[:, :, ig, :],
                        in_=y1t_ps.rearrange("p (c i) -> p c i", c=4))
                    if ig == 0:
                        # In^T batch for the group, issued after the group's
                        # first X load so it doesn't delay the weight stream
                        in_sb = in_pool.tile([128, GROUP, 128], F32)
                        nc.sync.dma_start(
                            out=in_sb,
                            in_=in_d[grp * GROUP:(grp + 1) * GROUP].rearrange(
                                "g j i -> j g i"))

                # pass 2 batched over the group: Y2T[j, (g i)]
                y2t_ps = y2_psum.tile([128, GROUP * 128], F32)
                for cc in range(4):
                    nc.tensor.matmul(
                        y2t_ps,
                        dt16[:, cc, :],
                        y1tb[:, cc, :, :].rearrange("p g i -> p (g i)"),
                        start=(cc == 0), stop=(cc == 3))

                diff_sb = diff_pool.tile([128, GROUP * 128], F32)
                nc.vector.tensor_sub(
                    diff_sb, y2t_ps,
                    in_sb.rearrange("p g i -> p (g i)"))
                nc.vector.tensor_reduce(
                    out=acc[:, grp:grp + 1], in_=diff_sb,
                    axis=mybir.AxisListType.X, op=mybir.AluOpType.add,
                    apply_absolute_value=True)

            nc.sync.dma_start(out=out_d.ap(), in_=acc)

    nc.compile()
    return nc


_PROGRAM = None


def _get_program():
    global _PROGRAM
    if _PROGRAM is None:
        _PROGRAM = _build_program()
    return _PROGRAM


def _shard_inputs(input, output):
    # fp16 quantization of X (identical numerics to an on-device cast)
    # + swizzle to [img, k_local, kc, c] for contiguous per-partition DMA.
    xs = np.asarray(output, dtype=np.float32).reshape(N_IMG, 4, 128, W)
    xs = np.ascontiguousarray(
        xs.transpose(0, 2, 1, 3)).astype(np.float16)
    # pre-transpose the small target images so the kernel compares Y2^T
    ins = np.ascontiguousarray(np.swapaxes(
        np.asarray(input, dtype=np.float32).reshape(N_IMG, h, w), 1, 2))
    in_maps = []
    for c in range(N_CORES):
        sl = slice(c * IMG_PER_CORE, (c + 1) * IMG_PER_CORE)
        in_maps.append({
            "x": np.ascontiguousarray(xs[sl]),
            "inp": np.ascontiguousarray(ins[sl]),
        })
    return in_maps


def _run(input, output, **kwargs):
    nc = _get_program()
    in_maps = _shard_inputs(input, output)
    res = run_bass_kernel_spmd(nc, in_maps, core_ids=list(range(N_CORES)),
                               **kwargs)
    total = 0.0
    for r in res.results:
        total += r["out"].astype(np.float64).sum()
    mean = total / float(N_IMG * h * w)
    return np.float32(mean), res


def kernel(input, output):
    val, _ = _run(input, output)
    return np.asarray(val, dtype=np.float32)
